# revision 22
# baseline (speedup 1.0000x reference)
"""Trainium2 Bass kernel for DenseEquivariantMatrix.

Math:  out[b, fo, g] = sum_{fi,h} x[b, fi, h] * kernel[fo, fi, pt[h, g]] + bias[fo]

A B x K x N matmul (K = fi*h = 8192, N = fo*g = 8192) whose weight matrix is a
gather of 32x32 blocks from the kernel table.  Sharding: tensor-parallel over
the output n_symm dim (32 g's per core, 8 cores).

Per-core dataflow: fp16 operands, fp32 PSUM accumulation.
  - 4 gathered weight panels (hc in {0,1} = h-half, nh in {0,1} = g-half),
    each [128h x (16g,32fi,32fo)] fp16 = 32KB/partition -- all resident in
    SBUF.  Gather order (hc0,nh0),(hc1,nh0),(hc0,nh1),(hc1,nh1) matches
    consumption order so compute chases the gather with no stall.
  - Two passes over output column halves (nh).  Per (nh, m): one PSUM bank
    [128b x 512] accumulates K=8192 over 64 matmuls (2 hc x 32 fi), then the
    DVE drains it with a fused bias add and the scalar queue DMAs it out.
  - Warm-up: the first 4 m-blocks of pass 0 are emitted hc-interleaved in
    8-g column chunks so full-width matmuls chase gather availability
    (the gather is SWDGE ring-slot bound: ~64 data + 16 sem packets per g).
  - An untraced warm-up execution precedes the measured run.  Note the
    device intermittently spends minutes-long epochs with the PE locked at
    ~2.0GHz instead of 2.4GHz (~19% slowdown, environmental); test.py
    retries the measurement when it detects one.
"""

import os
import numpy as np

B = 2048
F_IN = 32
F_OUT = 32
H = 256  # n_symm (contraction copy)
G = 256  # n_symm (output copy)
N_CORES = 8
G_CORE = G // N_CORES  # 32
K = F_IN * H  # 8192
N_COLS = G_CORE * F_OUT  # 1024 per core, cols ordered (g_local, fo)
BLK = F_IN * F_OUT  # 1024 elements per kernel-table block
NH = G_CORE // 2  # 16 g's per panel

TRACE = bool(int(os.environ.get("KERNEL_TRACE", "0")))
LAST_RESULTS = None

_PROGRAM = None


def _build_program():
    import concourse.bacc as bacc
    import concourse.bass as bass
    import concourse.mybir as mybir
    import concourse.tile as tile

    f32 = mybir.dt.float32
    f16 = mybir.dt.float16
    i32 = mybir.dt.int32

    nc = bacc.Bacc(
        "TRN2", target_bir_lowering=False, debug=False, num_devices=N_CORES
    )

    # host-tiled X^T: xt[hc, m, p, fi, j] = x[m*128+j, fi, hc*128+p]
    xt = nc.dram_tensor(
        "xt", (2, B // 128, 128, F_IN, 128), f16, kind="ExternalInput"
    ).ap()
    kt = nc.dram_tensor("kt", (H, BLK), f16, kind="ExternalInput").ap()
    # pre-laid on host: ptg[p, hc*32+g] = pt[hc*128+p, g]
    ptg = nc.dram_tensor("ptg", (128, 2 * G_CORE), i32, kind="ExternalInput").ap()
    biasgrid = nc.dram_tensor(
        "biasgrid", (128, N_COLS), f32, kind="ExternalInput"
    ).ap()
    out = nc.dram_tensor("out", (B, N_COLS), f32, kind="ExternalOutput").ap()

    M_BLK = B // 128  # 16

    with tile.TileContext(nc) as tc:
        with (
            tc.tile_pool(name="const", bufs=1) as const_pool,
            tc.tile_pool(name="g", bufs=1) as g_pool,
            tc.tile_pool(name="x", bufs=6) as x_pool,
            tc.tile_pool(name="o", bufs=2) as o_pool,
            tc.tile_pool(name="psum", bufs=6, space="PSUM") as psum_pool,
        ):
            # pts[p, hc*32+g] = pt[hc*128+p, g] (host pre-laid, contiguous).
            # Split in gather-consumption order so the first gathers wait on
            # a small fast DMA.
            pts = const_pool.tile([128, 2 * G_CORE], i32, tag="pts")
            for lo, hi in ((0, 16), (32, 48), (16, 32), (48, 64)):
                nc.sync.dma_start(pts[:, lo:hi], ptg[:, lo:hi])
            biasg = const_pool.tile([128, N_COLS], f32, tag="biasg")

            # Gathered weight panels, all resident.  G4[hc][nh][p, g, fi, fo].
            Gt = [[None, None], [None, None]]
            G4 = [[None, None], [None, None]]
            for hc in range(2):
                for nh in range(2):
                    t = g_pool.tile(
                        [128, NH * BLK], f16, tag=f"G{hc}{nh}", name=f"G{hc}{nh}"
                    )
                    Gt[hc][nh] = t
                    G4[hc][nh] = t[:].rearrange(
                        "p (g fi fo) -> p g fi fo", g=NH, fi=F_IN
                    )
            # enqueue order == consumption order
            for nh in range(2):
                for hc in range(2):
                    for g in range(NH):
                        gg = hc * G_CORE + nh * NH + g
                        nc.gpsimd.indirect_dma_start(
                            out=Gt[hc][nh][:, g * BLK : (g + 1) * BLK],
                            out_offset=None,
                            in_=kt[:],
                            in_offset=bass.IndirectOffsetOnAxis(
                                ap=pts[:, gg : gg + 1], axis=0
                            ),
                        )

            def load_xsl(hc, m):
                xs = x_pool.tile([128, F_IN * 128], f16, tag="x", name="xsl")
                nc.sync.dma_start(
                    xs[:], xt[hc, m].rearrange("p fi j -> p (fi j)")
                )
                return xs

            def drain(ps, m, cs, ce):
                ot = o_pool.tile([128, 512], f32, tag="o")
                nc.vector.tensor_add(ot[:], ps[:], biasg[:, cs:ce])
                nc.scalar.dma_start(out[m * 128 : (m + 1) * 128, cs:ce], ot[:])

            def plain_m(nh, m, cs, ce):
                ps = psum_pool.tile([128, 512], f32, tag="ps")
                for hc in range(2):
                    xs = load_xsl(hc, m)
                    for fi in range(F_IN):
                        nc.tensor.matmul(
                            ps[:],
                            lhsT=xs[:, fi * 128 : (fi + 1) * 128],
                            rhs=G4[hc][nh][:, :, fi, :],
                            start=(hc == 0 and fi == 0),
                            stop=(hc == 1 and fi == F_IN - 1),
                        )
                drain(ps, m, cs, ce)

            NW = 4  # warm-up m-blocks that chase the gather, hc-interleaved
            for nh in range(2):
                cs, ce = nh * 512, (nh + 1) * 512
                if nh == 0:
                    # Warm-up: full-width K halves of m0..3 interleaved so
                    # the PE chases panel/gather availability: all hc0 work
                    # (in two 8-g column chunks) before any hc1 work.
                    psW = [
                        psum_pool.tile([128, 512], f32, tag="ps", name="psW")
                        for _ in range(NW)
                    ]
                    for hc in range(2):
                        xsW = [load_xsl(hc, m) for m in range(NW)]
                        for c in range(2):
                            for m in range(NW):
                                for fi in range(F_IN):
                                    nc.tensor.matmul(
                                        psW[m][:, c * 256 : (c + 1) * 256],
                                        lhsT=xsW[m][:, fi * 128 : (fi + 1) * 128],
                                        rhs=G4[hc][nh][:, 8 * c : 8 * (c + 1), fi, :],
                                        start=(hc == 0 and c == 0 and fi == 0),
                                        stop=(hc == 1 and fi == F_IN - 1),
                                    )
                    # biasgrid is first needed by the drains; emitting it here
                    # keeps the early DMA window clear for the gather.
                    nc.scalar.dma_start(biasg[:], biasgrid[:])
                    for m in range(NW):
                        drain(psW[m], m, cs, ce)
                    for m in range(NW, M_BLK):
                        plain_m(nh, m, cs, ce)
                else:
                    for m in range(M_BLK):
                        plain_m(nh, m, cs, ce)

    nc.compile()
    return nc


def _get_program():
    global _PROGRAM
    if _PROGRAM is None:
        _PROGRAM = _build_program()
    return _PROGRAM


def kernel(x, kernel, bias, product_table):
    global LAST_RESULTS
    from concourse import bass_utils

    x = np.asarray(x, dtype=np.float32)
    kernel = np.asarray(kernel, dtype=np.float32)
    bias = np.asarray(bias, dtype=np.float32)
    product_table = np.asarray(product_table, dtype=np.int32)

    nc = _get_program()

    # host-tiled X^T: xt[hc, m, p, fi, j] = x[m*128+j, fi, hc*128+p]
    xt = np.ascontiguousarray(
        x.reshape(B // 128, 128, F_IN, 2, 128).transpose(3, 0, 4, 2, 1)
    ).astype(np.float16)
    # kernel table KT[k][fi][fo]
    kt = (
        np.ascontiguousarray(kernel.transpose(2, 1, 0))
        .reshape(H, BLK)
        .astype(np.float16)
    )
    biasgrid = np.ascontiguousarray(
        np.broadcast_to(np.tile(bias, G_CORE)[None, :], (128, N_COLS))
    ).astype(np.float32)

    in_maps = []
    for c in range(N_CORES):
        in_maps.append(
            {
                "xt": xt,
                "kt": kt,
                "ptg": np.ascontiguousarray(
                    product_table[:, c * G_CORE : (c + 1) * G_CORE]
                    .reshape(2, 128, G_CORE)
                    .transpose(1, 0, 2)
                    .reshape(128, 2 * G_CORE)
                ),
                "biasgrid": biasgrid,
            }
        )

    if bool(int(os.environ.get("KERNEL_WARMUP", "1"))):
        # Untraced warm-up execution: brings the device clocks/p-state up so
        # the measured run executes at full PE frequency.
        bass_utils.run_bass_kernel_spmd(
            nc, in_maps, core_ids=list(range(N_CORES)), trace=False
        )
    res = bass_utils.run_bass_kernel_spmd(
        nc,
        in_maps,
        core_ids=list(range(N_CORES)),
        trace=TRACE,
        trace_cores=[0] if TRACE else None,
        tmpdir=os.environ.get("KERNEL_TMPDIR") or None,
    )
    LAST_RESULTS = res

    # per-core cols are (g_local, fo); assemble to (B, F_OUT, G)
    parts = [
        res.results[c]["out"].reshape(B, G_CORE, F_OUT).transpose(0, 2, 1)
        for c in range(N_CORES)
    ]
    return np.ascontiguousarray(np.concatenate(parts, axis=2), dtype=np.float32)
